# revision 31
# baseline (speedup 1.0000x reference)
"""AFNO2D Bass kernel for 8 TRN2 NeuronCores.

Sharding: core k handles batch b=k//4 and channel group g=k%4, i.e. channels
[192*g, 192*(g+1)) = spectral-MLP blocks {2g, 2g+1}. The whole pipeline
(2D DHT -> block-diagonal MLP -> inverse DHT -> residual) is channel-local
under this sharding, so there are no collectives.

Math (per channel image U [128,128]; A=cos, B=sin, Ch=A+B, D=A-B, 128x128
symmetric; n = 16384):
  Xk = DHT2D(U) = (A@U)@Ch + (B@U)@D
  flip_periodic = reversal of the flattened spatial tail (p -> (-p) mod n)
  MLP with flip commuted through relu/matmul:
     o2 = relu(Xk@W1k)@W20 + flip(relu(Xk@W1nk)@(W20+W21))
  y = (Ch@(o2@A) + D@(o2@B)) / n + x

On-chip memory is managed as one SBUF "arena" tile whose regions are
time-shared across stages; Tile's interval-level dependency tracking turns
region reuse into the right WAR/RAW ordering.
"""

import os
import numpy as np
import ml_dtypes

import concourse.bass as bass
import concourse.bacc as bacc
import concourse.mybir as mybir
import concourse.tile as tile
import concourse.bass_utils as bass_utils

BF16 = mybir.dt.bfloat16
F32 = mybir.dt.float32

H = 128
W = 128
NPOS = H * W
CG = 192          # channels per core (2 blocks of 96)
BS = 96           # MLP block size
NCORES = 8

_DEBUG = bool(int(os.environ.get("AFNO_DEBUG", "0")))

# arena regions (elements per partition, bf16)
ARENA_E = 92160
REG_R12T = (0, 49152)          # P1 out / P2 in, [w, k1*384+blk*192+br*96+c]
REG_XSB = (49152, 73728)       # x image, P1 in
REG_XK = (49152, 81920)        # Xk [blk1 | blk0], P2 out / P3 in
REG_F1 = (0, 16384)            # per-blk relu(Xk@W1k)
REG_EF = (16384, 32768)        # per-blk flip(relu(Xk@W1nk))
REG_O2T = (65536, 90112)       # o2 transposed [k2, k1*192+c]; later ybuf
REG_N12 = (0, 49152)           # (V@A | V@B) per channel


def _build_graph():
    nc = bacc.Bacc("TRN2", target_bir_lowering=False, debug=False)
    COPY = mybir.ActivationFunctionType.Copy
    RELU_ = mybir.ActivationFunctionType.Relu

    x_ext = nc.declare_dram_parameter("x", [NPOS, CG], BF16, isOutput=False)
    xt_ext = nc.declare_dram_parameter("xt", [CG, NPOS], BF16, isOutput=False)
    consts_ext = nc.declare_dram_parameter("consts", [128, 768], BF16, isOutput=False)
    wts_ext = nc.declare_dram_parameter("wts", [BS, 8 * BS], BF16, isOutput=False)
    # output is channel-major; the host transposes back
    out_ext = nc.declare_dram_parameter("out", [CG, NPOS], BF16, isOutput=True)

    dbg = {}
    if _DEBUG:
        for name, shape in [
            ("dbg_r12t", [128, CG * 256]),
            ("dbg_xk", [BS, 2 * NPOS]),
            ("dbg_f1", [BS, 2 * NPOS]),
            ("dbg_ef", [BS, 2 * NPOS]),
            ("dbg_o2t", [128, 128 * CG]),
            ("dbg_n12", [128, CG * 256]),
        ]:
            dbg[name] = nc.declare_dram_parameter(name, shape, BF16, isOutput=True)

    with tile.TileContext(nc) as tc:
        with tc.tile_pool(name="sb", bufs=1) as sb:
            consts_sb = sb.tile([128, 768], BF16)
            wts_sb = sb.tile([BS, 8 * BS], BF16)
            arena = sb.tile([128, ARENA_E], BF16)

            nc.sync.dma_start(consts_sb[:], consts_ext[:])
            nc.sync.dma_start(wts_sb[:], wts_ext[:])
            CAB = consts_sb[:, 0:256]
            CH = consts_sb[:, 256:384]
            CD = consts_sb[:, 384:512]
            CHN = consts_sb[:, 512:640]
            CDN = consts_sb[:, 640:768]

            def wslice(blk, j):  # j: 0=W1k 1=W1nk 2=W20 3=W20+W21
                o = (blk * 4 + j) * BS
                return wts_sb[:, o:o + BS]

            def reg(r, rows=128):
                return arena[0:rows, r[0]:r[1]]

            # ------------- Stage P1:  r12t[w, c*256 + br*128 + k1] --------
            # (unit-stride evacuations; P2 pays with strided LDWEIGHTS)
            r12t = reg(REG_R12T)
            # x stored as two channel-half regions so P1 on half 0 can start
            # as soon as the first DMA lands
            xh = [arena[:, 49152:61440], arena[:, 61440:73728]]
            xe3 = x_ext.rearrange("(h w) c -> h w c", w=W)
            for hf in range(2):
                nc.sync.dma_start(
                    xh[hf].rearrange("p (w c) -> p w c", c=BS),
                    xe3[:, :, hf * BS:(hf + 1) * BS])
            with tc.tile_pool(name="pp1", bufs=6, space="PSUM") as pp1:
                for cp in range(CG // 2):
                    ps = pp1.tile([128, 512], F32, tag="p1", name="ps_p1")
                    for u in range(2):
                        c = 2 * cp + u
                        nc.tensor.matmul(ps[:, u * 256:(u + 1) * 256],
                                         xh[c // BS][:, (c % BS)::BS], CAB,
                                         start=True, stop=True)
                    dst = r12t[:, cp * 512:(cp + 1) * 512]
                    if cp % 2 == 0:
                        nc.scalar.activation(dst, ps[:], COPY)
                    else:
                        nc.vector.tensor_copy(dst, ps[:])
            if _DEBUG:
                nc.sync.dma_start(dbg["dbg_r12t"][:], r12t)

            # ------------- Stage P2:  xk[c, blk, k1*128+k2] ---------------
            # storage order [blk1 | blk0] so blk0 (processed first in P3/P4T)
            # sits in the region later overwritten by o2t.
            xkr = reg(REG_XK, rows=BS)
            xk3 = xkr.rearrange("p (b q) -> p b q", b=2)  # b=0 -> blk1

            def xk_blk(blk):
                return xk3[:, 1 - blk]

            # hop2: re-layout k1-chunks of r12t into a contiguous-LS ring so
            # P2's LDWEIGHTS are contiguous. DVE strided-reads are ~1cyc/elem
            # (ACT ~2), contiguous writes full speed.
            r12q = r12t.rearrange("p (c b q) -> p q b c", b=2, q=128)
            KC = 8  # k1 per chunk
            with tc.tile_pool(name="pp2", bufs=6, space="PSUM") as pp2:
                for j in range(128 // KC):
                    slot = 81920 + (j % 3) * (KC * 384)
                    ring = arena[:, slot:slot + KC * 384]
                    rv = ring.rearrange("p (q b c) -> p q b c", b=2, c=CG)
                    # split per channel-half: the first halves only depend on
                    # P1's first 96 channels, so hop2 overlaps P1
                    for hf in range(2):
                        src = r12q[:, j * KC:(j + 1) * KC, :,
                                   hf * BS:(hf + 1) * BS]
                        dv = rv[:, :, :, hf * BS:(hf + 1) * BS]
                        if (2 * j + hf) % 3 == 2:
                            nc.scalar.activation(dv, src, COPY)
                        else:
                            nc.vector.tensor_copy(dv, src)
                    for kl in range(KC):
                        k1 = j * KC + kl
                        ps = pp2.tile([BS, 256], F32, tag="p2", name="ps_p2")
                        for blk in (1, 0):
                            base = slot + kl * 384 + blk * BS
                            sl = ps[:, (1 - blk) * 128:(2 - blk) * 128]
                            nc.tensor.matmul(sl, arena[0:128, base:base + BS],
                                             CH, start=True, stop=False)
                            nc.tensor.matmul(
                                sl, arena[0:128, base + CG:base + CG + BS],
                                CD, start=False, stop=True)
                        dst = xk3[:, :, k1 * 128:(k1 + 1) * 128]
                        if k1 % 2 == 0:
                            nc.vector.tensor_copy(dst, ps[:])
                        else:
                            nc.scalar.activation(dst, ps[:], COPY)
            if _DEBUG:
                # dbg_xk layout [c, blk0 | blk1]
                nc.sync.dma_start(dbg["dbg_xk"][:, 0:NPOS], xk_blk(0))
                nc.sync.dma_start(dbg["dbg_xk"][:, NPOS:2 * NPOS], xk_blk(1))

            # ------------- Stage P3 + P4T (per block) ---------------------
            o2t = reg(REG_O2T)
            with tc.tile_pool(name="pp34", bufs=4, space="PSUM") as pp34:
                for blk in range(2):
                    # f1 double-buffered across blocks so P3(blk1) evacs don't
                    # wait for P4T(blk0); ef shares one slot.
                    f1 = arena[0:BS, 32768 * blk:32768 * blk + 16384]
                    ef = reg(REG_EF, rows=BS)
                    xkb = xk_blk(blk)
                    for br in range(2):
                        for ch in range(NPOS // 512):
                            ps = pp34.tile([BS, 512], F32, tag="p3", name="ps_p3", bufs=3)
                            nc.tensor.matmul(ps[:], wslice(blk, br),
                                             xkb[:, ch * 512:(ch + 1) * 512],
                                             start=True, stop=True)
                            # evacuation split across ACT+DVE so it keeps up
                            if br == 0:
                                o = ch * 512
                                nc.scalar.activation(
                                    f1[:, o:o + 256], ps[:, 0:256], RELU_)
                                nc.vector.tensor_scalar_max(
                                    f1[:, o + 256:o + 512], ps[:, 256:512], 0.0)
                            else:
                                e = NPOS - 512 * ch
                                if ch == 0:
                                    nc.vector.tensor_scalar_max(
                                        ef[:, 0:1], ps[:, 0:1], 0.0)
                                    nc.vector.tensor_scalar_max(
                                        ef[:, NPOS - 1:NPOS - 256:-1],
                                        ps[:, 1:256], 0.0)
                                else:
                                    nc.vector.tensor_scalar_max(
                                        ef[:, e:e - 256:-1], ps[:, 0:256], 0.0)
                                nc.scalar.activation(
                                    ef[:, e - 256:e - 512:-1],
                                    ps[:, 256:512], RELU_)
                    if _DEBUG:
                        nc.sync.dma_start(
                            dbg["dbg_f1"][:, blk * NPOS:(blk + 1) * NPOS], f1)
                        nc.sync.dma_start(
                            dbg["dbg_ef"][:, blk * NPOS:(blk + 1) * NPOS], ef)
                    for k1p in range(64):
                        ps = pp34.tile([128, 2 * BS], F32, tag="p4", name="ps_p4", bufs=5)
                        for g in range(2):
                            k1 = 2 * k1p + g
                            sl = ps[:, g * BS:(g + 1) * BS]
                            nc.tensor.matmul(sl,
                                             f1[:, k1 * 128:(k1 + 1) * 128],
                                             wslice(blk, 2),
                                             start=True, stop=False)
                            nc.tensor.matmul(sl,
                                             ef[:, k1 * 128:(k1 + 1) * 128],
                                             wslice(blk, 3),
                                             start=False, stop=True)
                        dst = o2t.rearrange("p (q v) -> p q v", v=CG)[
                            :, 2 * k1p:2 * k1p + 2, blk * BS:blk * BS + BS]
                        if k1p % 2 == 0:
                            nc.scalar.activation(dst, ps[:], COPY)
                        else:
                            nc.vector.tensor_copy(dst, ps[:])
            if _DEBUG:
                nc.sync.dma_start(dbg["dbg_o2t"][:], o2t)

            # ------------- Stage S3a':  n12[k1, c*256+(wA|wB)] ------------
            n12 = reg(REG_N12)
            with tc.tile_pool(name="pp5", bufs=4, space="PSUM") as pp5:
                for cp in range(CG // 2):
                    ps = pp5.tile([128, 512], F32, tag="p5", name="ps_p5")
                    for u in range(2):
                        cs = 2 * cp + u
                        nc.tensor.matmul(ps[:, u * 256:(u + 1) * 256],
                                         o2t[:, cs::CG], CAB,
                                         start=True, stop=True)
                    dst = n12[:, cp * 512:(cp + 1) * 512]
                    if cp % 2 == 0:
                        nc.vector.tensor_copy(dst, ps[:])
                    else:
                        nc.scalar.activation(dst, ps[:], COPY)
            if _DEBUG:
                nc.sync.dma_start(dbg["dbg_n12"][:], n12)

            # ------------- Stage S4': y = (Ch@N1 + D@N2)/n + x ------------
            # channel-chunked; ybuf is channel-major [h, c*128+w] so psum
            # evacuations are plain unit-stride; output DRAM is channel-major
            # (host transposes back) and bias comes from the xt input.
            n12v = n12.rearrange("p (c x) -> p c x", x=256)   # [128, 192, 256]
            ybuf = reg(REG_O2T)                               # reuse o2t region
            xtv = xt_ext.rearrange("c (h w) -> h c w", w=W)   # [h, CG, 128]
            otv = out_ext.rearrange("c (h w) -> h c w", w=W)
            # bias strips prefetched into 2 rotating slots (region frees once
            # P3 is done); add+store interleaved per 48-channel range
            def bias_dma(cr):
                boff = 49152 + (cr % 4) * 3072
                nc.sync.dma_start(
                    arena[:, boff:boff + 3072].rearrange("p (c w) -> p c w", w=W),
                    xtv[:, cr * 24:cr * 24 + 24, :])

            for _cr in range(4):
                bias_dma(_cr)
            with tc.tile_pool(name="pp6", bufs=4, space="PSUM") as pp6:
                for ck in range(CG // 4):
                    c0 = 4 * ck
                    ps = pp6.tile([128, 512], F32, tag="p6", name="ps_p6")
                    nc.tensor.matmul(ps[:], CHN, n12v[:, c0:c0 + 4, 0:128],
                                     start=True, stop=False)
                    nc.tensor.matmul(ps[:], CDN, n12v[:, c0:c0 + 4, 128:256],
                                     start=False, stop=True)
                    dst = ybuf[:, c0 * 128:(c0 + 4) * 128]
                    if ck % 2 == 0:
                        nc.scalar.activation(dst, ps[:], COPY)
                    else:
                        nc.vector.tensor_copy(dst, ps[:])
                    if ck % 6 == 5:
                        cr = ck // 6
                        boff = 49152 + (cr % 4) * 3072
                        bias = arena[:, boff:boff + 3072]
                        yslice = ybuf[:, cr * 24 * 128:(cr + 1) * 24 * 128]
                        nc.vector.tensor_add(yslice, yslice, bias)
                        nc.sync.dma_start(
                            otv[:, cr * 24:cr * 24 + 24, :],
                            yslice.rearrange("p (c w) -> p c w", w=W))
                        if cr + 4 < 8:
                            bias_dma(cr + 4)

    nc.finalize()
    return nc


_NC_CACHE = None


def _get_graph():
    global _NC_CACHE
    if _NC_CACHE is None:
        _NC_CACHE = _build_graph()
    return _NC_CACHE


def _host_constants():
    k = np.arange(128)
    th = 2.0 * np.pi * np.outer(k, k) / 128.0
    A = np.cos(th)
    B = np.sin(th)
    Ch = A + B
    D = A - B
    n = float(NPOS)
    consts = np.concatenate(
        [A, B, Ch, D, Ch / n, D / n], axis=1).astype(np.float32)
    return consts.astype(ml_dtypes.bfloat16)


def kernel(x, w1, w2):
    x = np.asarray(x, dtype=np.float32)
    w1 = np.asarray(w1, dtype=np.float32)
    w2 = np.asarray(w2, dtype=np.float32)
    assert x.shape == (2, NPOS, 768)

    consts = _host_constants()
    xbf = x.astype(ml_dtypes.bfloat16)

    in_maps = []
    for core in range(NCORES):
        b, g = core // 4, core % 4
        xs = np.ascontiguousarray(xbf[b, :, CG * g:CG * (g + 1)])
        wts = np.empty((BS, 8 * BS), np.float32)
        for blk2 in range(2):
            blk = 2 * g + blk2
            o = blk2 * 4 * BS
            wts[:, o + 0 * BS:o + 1 * BS] = w1[0, blk]
            wts[:, o + 1 * BS:o + 2 * BS] = w1[1, blk]
            wts[:, o + 2 * BS:o + 3 * BS] = w2[0, blk]
            wts[:, o + 3 * BS:o + 4 * BS] = w2[0, blk] + w2[1, blk]
        in_maps.append({
            "x": xs,
            "xt": np.ascontiguousarray(xs.T),
            "consts": consts,
            "wts": wts.astype(ml_dtypes.bfloat16),
        })

    nc = _get_graph()
    trace = bool(int(os.environ.get("AFNO_TRACE", "0")))
    res = bass_utils.run_bass_kernel_spmd(
        nc, in_maps, list(range(NCORES)), trace=trace)
    kernel.last_result = res

    y = np.empty((2, NPOS, 768), np.float32)
    for core in range(NCORES):
        b, g = core // 4, core % 4
        y[b, :, CG * g:CG * (g + 1)] = res.results[core]["out"].T.astype(np.float32)
    return y


# revision 32
# speedup vs baseline: 1.0880x; 1.0880x over previous
"""AFNO2D Bass kernel for 8 TRN2 NeuronCores.

Sharding: core k handles batch b=k//4 and channel group g=k%4, i.e. channels
[192*g, 192*(g+1)) = spectral-MLP blocks {2g, 2g+1}. The whole pipeline
(2D DHT -> block-diagonal MLP -> inverse DHT -> residual) is channel-local
under this sharding, so there are no collectives.

Math (per channel image U [128,128]; A=cos, B=sin, Ch=A+B, D=A-B, 128x128
symmetric; n = 16384):
  Xk = DHT2D(U) = (A@U)@Ch + (B@U)@D
  flip_periodic = reversal of the flattened spatial tail (p -> (-p) mod n)
  MLP with flip commuted through relu/matmul:
     o2 = relu(Xk@W1k)@W20 + flip(relu(Xk@W1nk)@(W20+W21))
  y = (Ch@(o2@A) + D@(o2@B)) / n + x

On-chip memory is managed as one SBUF "arena" tile whose regions are
time-shared across stages; Tile's interval-level dependency tracking turns
region reuse into the right WAR/RAW ordering.
"""

import os
import numpy as np
import ml_dtypes

import concourse.bass as bass
import concourse.bacc as bacc
import concourse.mybir as mybir
import concourse.tile as tile
import concourse.bass_utils as bass_utils

BF16 = mybir.dt.bfloat16
F32 = mybir.dt.float32

H = 128
W = 128
NPOS = H * W
CG = 192          # channels per core (2 blocks of 96)
BS = 96           # MLP block size
NCORES = 8

_DEBUG = bool(int(os.environ.get("AFNO_DEBUG", "0")))

# arena regions (elements per partition, bf16)
ARENA_E = 92160
REG_R12T = (0, 49152)          # P1 out / P2 in, [w, k1*384+blk*192+br*96+c]
REG_XSB = (49152, 73728)       # x image, P1 in
REG_XK = (49152, 81920)        # Xk [blk1 | blk0], P2 out / P3 in
REG_F1 = (0, 16384)            # per-blk relu(Xk@W1k)
REG_EF = (16384, 32768)        # per-blk flip(relu(Xk@W1nk))
REG_O2T = (65536, 90112)       # o2 transposed [k2, k1*192+c]; later ybuf
REG_N12 = (0, 49152)           # (V@A | V@B) per channel


def _build_graph():
    nc = bacc.Bacc("TRN2", target_bir_lowering=False, debug=False)
    COPY = mybir.ActivationFunctionType.Copy
    RELU_ = mybir.ActivationFunctionType.Relu

    x_ext = nc.declare_dram_parameter("x", [NPOS, CG], BF16, isOutput=False)
    xt_ext = nc.declare_dram_parameter("xt", [CG, NPOS], BF16, isOutput=False)
    consts_ext = nc.declare_dram_parameter("consts", [128, 768], BF16, isOutput=False)
    wts_ext = nc.declare_dram_parameter("wts", [BS, 8 * BS], BF16, isOutput=False)
    # output is channel-major; the host transposes back
    out_ext = nc.declare_dram_parameter("out", [CG, NPOS], BF16, isOutput=True)

    dbg = {}
    if _DEBUG:
        for name, shape in [
            ("dbg_r12t", [128, CG * 256]),
            ("dbg_xk", [BS, 2 * NPOS]),
            ("dbg_f1", [BS, 2 * NPOS]),
            ("dbg_ef", [BS, 2 * NPOS]),
            ("dbg_o2t", [128, 128 * CG]),
            ("dbg_n12", [128, CG * 256]),
        ]:
            dbg[name] = nc.declare_dram_parameter(name, shape, BF16, isOutput=True)

    with tile.TileContext(nc) as tc:
        with tc.tile_pool(name="sb", bufs=1) as sb:
            consts_sb = sb.tile([128, 768], BF16)
            wts_sb = sb.tile([BS, 8 * BS], BF16)
            arena = sb.tile([128, ARENA_E], BF16)

            nc.sync.dma_start(consts_sb[:], consts_ext[:])
            nc.sync.dma_start(wts_sb[:], wts_ext[:])
            CAB = consts_sb[:, 0:256]
            CH = consts_sb[:, 256:384]
            CD = consts_sb[:, 384:512]
            CHN = consts_sb[:, 512:640]
            CDN = consts_sb[:, 640:768]

            def wslice(blk, j):  # j: 0=W1k 1=W1nk 2=W20 3=W20+W21
                o = (blk * 4 + j) * BS
                return wts_sb[:, o:o + BS]

            def reg(r, rows=128):
                return arena[0:rows, r[0]:r[1]]

            # ------------- Stage P1:  r12t[w, c*256 + br*128 + k1] --------
            # (unit-stride evacuations; P2 pays with strided LDWEIGHTS)
            r12t = reg(REG_R12T)
            # x stored as two channel-half regions so P1 on half 0 can start
            # as soon as the first DMA lands
            xh = [arena[:, 49152:61440], arena[:, 61440:73728]]
            xe3 = x_ext.rearrange("(h w) c -> h w c", w=W)
            for hf in range(2):
                nc.sync.dma_start(
                    xh[hf].rearrange("p (w c) -> p w c", c=BS),
                    xe3[:, :, hf * BS:(hf + 1) * BS])
            with tc.tile_pool(name="pp1", bufs=6, space="PSUM") as pp1:
                for cp in range(CG // 2):
                    ps = pp1.tile([128, 512], F32, tag="p1", name="ps_p1")
                    for u in range(2):
                        c = 2 * cp + u
                        nc.tensor.matmul(ps[:, u * 256:(u + 1) * 256],
                                         xh[c // BS][:, (c % BS)::BS], CAB,
                                         start=True, stop=True)
                    dst = r12t[:, cp * 512:(cp + 1) * 512]
                    if cp % 2 == 0:
                        nc.scalar.activation(dst, ps[:], COPY)
                    else:
                        nc.vector.tensor_copy(dst, ps[:])
            if _DEBUG:
                nc.sync.dma_start(dbg["dbg_r12t"][:], r12t)

            # ------------- Stage P2:  xk[c, blk, k1*128+k2] ---------------
            # storage order [blk1 | blk0] so blk0 (processed first in P3/P4T)
            # sits in the region later overwritten by o2t.
            xkr = reg(REG_XK, rows=BS)
            xk3 = xkr.rearrange("p (b q) -> p b q", b=2)  # b=0 -> blk1

            def xk_blk(blk):
                return xk3[:, 1 - blk]

            # hop2: re-layout k1-chunks of r12t into a contiguous-LS ring so
            # P2's LDWEIGHTS are contiguous. DVE strided-reads are ~1cyc/elem
            # (ACT ~2), contiguous writes full speed.
            r12q = r12t.rearrange("p (c b q) -> p q b c", b=2, q=128)
            KC = 8  # k1 per chunk
            with tc.tile_pool(name="pp2", bufs=6, space="PSUM") as pp2:
                for j in range(128 // KC):
                    slot = 81920 + (j % 3) * (KC * 384)
                    ring = arena[:, slot:slot + KC * 384]
                    rv = ring.rearrange("p (q b c) -> p q b c", b=2, c=CG)
                    src = r12q[:, j * KC:(j + 1) * KC, :, :]
                    if j % 3 == 2:
                        nc.scalar.activation(rv, src, COPY)
                    else:
                        nc.vector.tensor_copy(rv, src)
                    for kl in range(KC):
                        k1 = j * KC + kl
                        ps = pp2.tile([BS, 256], F32, tag="p2", name="ps_p2")
                        for blk in (1, 0):
                            base = slot + kl * 384 + blk * BS
                            sl = ps[:, (1 - blk) * 128:(2 - blk) * 128]
                            nc.tensor.matmul(sl, arena[0:128, base:base + BS],
                                             CH, start=True, stop=False)
                            nc.tensor.matmul(
                                sl, arena[0:128, base + CG:base + CG + BS],
                                CD, start=False, stop=True)
                        dst = xk3[:, :, k1 * 128:(k1 + 1) * 128]
                        if k1 % 2 == 0:
                            nc.vector.tensor_copy(dst, ps[:])
                        else:
                            nc.scalar.activation(dst, ps[:], COPY)
            if _DEBUG:
                # dbg_xk layout [c, blk0 | blk1]
                nc.sync.dma_start(dbg["dbg_xk"][:, 0:NPOS], xk_blk(0))
                nc.sync.dma_start(dbg["dbg_xk"][:, NPOS:2 * NPOS], xk_blk(1))

            # ------------- Stage P3 + P4T (per block) ---------------------
            o2t = reg(REG_O2T)
            with tc.tile_pool(name="pp34", bufs=4, space="PSUM") as pp34:
                for blk in range(2):
                    # f1 double-buffered across blocks so P3(blk1) evacs don't
                    # wait for P4T(blk0); ef shares one slot.
                    f1 = arena[0:BS, 32768 * blk:32768 * blk + 16384]
                    ef = reg(REG_EF, rows=BS)
                    xkb = xk_blk(blk)
                    for br in range(2):
                        for ch in range(NPOS // 512):
                            ps = pp34.tile([BS, 512], F32, tag="p3", name="ps_p3")
                            nc.tensor.matmul(ps[:], wslice(blk, br),
                                             xkb[:, ch * 512:(ch + 1) * 512],
                                             start=True, stop=True)
                            # evacuation split across ACT+DVE so it keeps up
                            if br == 0:
                                o = ch * 512
                                nc.scalar.activation(
                                    f1[:, o:o + 256], ps[:, 0:256], RELU_)
                                nc.vector.tensor_scalar_max(
                                    f1[:, o + 256:o + 512], ps[:, 256:512], 0.0)
                            else:
                                e = NPOS - 512 * ch
                                if ch == 0:
                                    nc.vector.tensor_scalar_max(
                                        ef[:, 0:1], ps[:, 0:1], 0.0)
                                    nc.vector.tensor_scalar_max(
                                        ef[:, NPOS - 1:NPOS - 256:-1],
                                        ps[:, 1:256], 0.0)
                                else:
                                    nc.vector.tensor_scalar_max(
                                        ef[:, e:e - 256:-1], ps[:, 0:256], 0.0)
                                nc.scalar.activation(
                                    ef[:, e - 256:e - 512:-1],
                                    ps[:, 256:512], RELU_)
                    if _DEBUG:
                        nc.sync.dma_start(
                            dbg["dbg_f1"][:, blk * NPOS:(blk + 1) * NPOS], f1)
                        nc.sync.dma_start(
                            dbg["dbg_ef"][:, blk * NPOS:(blk + 1) * NPOS], ef)
                    for k1p in range(64):
                        ps = pp34.tile([128, 2 * BS], F32, tag="p4", name="ps_p4")
                        for g in range(2):
                            k1 = 2 * k1p + g
                            sl = ps[:, g * BS:(g + 1) * BS]
                            nc.tensor.matmul(sl,
                                             f1[:, k1 * 128:(k1 + 1) * 128],
                                             wslice(blk, 2),
                                             start=True, stop=False)
                            nc.tensor.matmul(sl,
                                             ef[:, k1 * 128:(k1 + 1) * 128],
                                             wslice(blk, 3),
                                             start=False, stop=True)
                        dst = o2t.rearrange("p (q v) -> p q v", v=CG)[
                            :, 2 * k1p:2 * k1p + 2, blk * BS:blk * BS + BS]
                        if k1p % 2 == 0:
                            nc.scalar.activation(dst, ps[:], COPY)
                        else:
                            nc.vector.tensor_copy(dst, ps[:])
            if _DEBUG:
                nc.sync.dma_start(dbg["dbg_o2t"][:], o2t)

            # ------------- Stage S3a':  n12[k1, c*256+(wA|wB)] ------------
            n12 = reg(REG_N12)
            with tc.tile_pool(name="pp5", bufs=4, space="PSUM") as pp5:
                for cp in range(CG // 2):
                    ps = pp5.tile([128, 512], F32, tag="p5", name="ps_p5")
                    for u in range(2):
                        cs = 2 * cp + u
                        nc.tensor.matmul(ps[:, u * 256:(u + 1) * 256],
                                         o2t[:, cs::CG], CAB,
                                         start=True, stop=True)
                    dst = n12[:, cp * 512:(cp + 1) * 512]
                    if cp % 2 == 0:
                        nc.vector.tensor_copy(dst, ps[:])
                    else:
                        nc.scalar.activation(dst, ps[:], COPY)
            if _DEBUG:
                nc.sync.dma_start(dbg["dbg_n12"][:], n12)

            # ------------- Stage S4': y = (Ch@N1 + D@N2)/n + x ------------
            # channel-chunked; ybuf is channel-major [h, c*128+w] so psum
            # evacuations are plain unit-stride; output DRAM is channel-major
            # (host transposes back) and bias comes from the xt input.
            n12v = n12.rearrange("p (c x) -> p c x", x=256)   # [128, 192, 256]
            ybuf = reg(REG_O2T)                               # reuse o2t region
            xtv = xt_ext.rearrange("c (h w) -> h c w", w=W)   # [h, CG, 128]
            otv = out_ext.rearrange("c (h w) -> h c w", w=W)
            # bias strips prefetched into 2 rotating slots (region frees once
            # P3 is done); add+store interleaved per 48-channel range
            def bias_dma(cr):
                boff = 49152 + (cr % 2) * 6144
                nc.sync.dma_start(
                    arena[:, boff:boff + 6144].rearrange("p (c w) -> p c w", w=W),
                    xtv[:, cr * 48:cr * 48 + 48, :])

            bias_dma(0)
            bias_dma(1)
            with tc.tile_pool(name="pp6", bufs=4, space="PSUM") as pp6:
                for ck in range(CG // 4):
                    c0 = 4 * ck
                    ps = pp6.tile([128, 512], F32, tag="p6", name="ps_p6")
                    nc.tensor.matmul(ps[:], CHN, n12v[:, c0:c0 + 4, 0:128],
                                     start=True, stop=False)
                    nc.tensor.matmul(ps[:], CDN, n12v[:, c0:c0 + 4, 128:256],
                                     start=False, stop=True)
                    dst = ybuf[:, c0 * 128:(c0 + 4) * 128]
                    if ck % 2 == 0:
                        nc.scalar.activation(dst, ps[:], COPY)
                    else:
                        nc.vector.tensor_copy(dst, ps[:])
                    if ck % 12 == 11:
                        cr = ck // 12
                        boff = 49152 + (cr % 2) * 6144
                        bias = arena[:, boff:boff + 6144]
                        yslice = ybuf[:, cr * 48 * 128:(cr + 1) * 48 * 128]
                        nc.vector.tensor_add(yslice, yslice, bias)
                        nc.sync.dma_start(
                            otv[:, cr * 48:cr * 48 + 48, :],
                            yslice.rearrange("p (c w) -> p c w", w=W))
                        if cr + 2 < 4:
                            bias_dma(cr + 2)

    nc.finalize()
    return nc


_NC_CACHE = None


def _get_graph():
    global _NC_CACHE
    if _NC_CACHE is None:
        _NC_CACHE = _build_graph()
    return _NC_CACHE


def _host_constants():
    k = np.arange(128)
    th = 2.0 * np.pi * np.outer(k, k) / 128.0
    A = np.cos(th)
    B = np.sin(th)
    Ch = A + B
    D = A - B
    n = float(NPOS)
    consts = np.concatenate(
        [A, B, Ch, D, Ch / n, D / n], axis=1).astype(np.float32)
    return consts.astype(ml_dtypes.bfloat16)


def kernel(x, w1, w2):
    x = np.asarray(x, dtype=np.float32)
    w1 = np.asarray(w1, dtype=np.float32)
    w2 = np.asarray(w2, dtype=np.float32)
    assert x.shape == (2, NPOS, 768)

    consts = _host_constants()
    xbf = x.astype(ml_dtypes.bfloat16)

    in_maps = []
    for core in range(NCORES):
        b, g = core // 4, core % 4
        xs = np.ascontiguousarray(xbf[b, :, CG * g:CG * (g + 1)])
        wts = np.empty((BS, 8 * BS), np.float32)
        for blk2 in range(2):
            blk = 2 * g + blk2
            o = blk2 * 4 * BS
            wts[:, o + 0 * BS:o + 1 * BS] = w1[0, blk]
            wts[:, o + 1 * BS:o + 2 * BS] = w1[1, blk]
            wts[:, o + 2 * BS:o + 3 * BS] = w2[0, blk]
            wts[:, o + 3 * BS:o + 4 * BS] = w2[0, blk] + w2[1, blk]
        in_maps.append({
            "x": xs,
            "xt": np.ascontiguousarray(xs.T),
            "consts": consts,
            "wts": wts.astype(ml_dtypes.bfloat16),
        })

    nc = _get_graph()
    trace = bool(int(os.environ.get("AFNO_TRACE", "0")))
    res = bass_utils.run_bass_kernel_spmd(
        nc, in_maps, list(range(NCORES)), trace=trace)
    kernel.last_result = res

    y = np.empty((2, NPOS, 768), np.float32)
    for core in range(NCORES):
        b, g = core // 4, core % 4
        y[b, :, CG * g:CG * (g + 1)] = res.results[core]["out"].T.astype(np.float32)
    return y
